# revision 4
# baseline (speedup 1.0000x reference)
"""nn_MoE_57492432224434 — MoE (SwiGLU, top-2 of 8 experts) on 8 TRN2 NeuronCores.

Expert-parallel: host routes tokens (tiny gate matmul + top-2 + softmax) and
dispatches expert e's tokens to core e, transposed and zero-padded to the max
expert load C. Each core runs one expert's SwiGLU in bf16 (fp32 PSUM accum):

  phase A: gT = silu((x @ w1.T).T) * (x @ w2.T).T      [PE + ACT + DVE]
  phase B: yT = (g @ w3.T).T * combine_weight           [PE + DVE]

Perf notes (measured on hw):
  * capacity = exact max expert load (no rounding), even token tiles >=256
    wide, <=512 (PSUM bank limit; odd widths fail the matmul ISA check)
  * bf16 weights/activations: Ldweights+Matmult pairs let the PE reorder
    window hide weight loads; DMA halves; rel err ~4e-3 (tolerance 2e-2)
  * x is loaded as (k-chunk, token-tile) tiles t-major so the PE starts ~3us
    in; weight DMAs ride the Act HWDGE queue, x/outputs the SP queue
  * phase A k-outer/t-inner (consecutive matmuls share the stationary block),
    f=0 t-outer to chase the x DMAs
  * next call's x prefetches during phase B (pipelines back-to-back calls)
  * host scatter-adds each core's yT columns into the final [T, D] output
"""
import numpy as np
import orjson

import concourse.bass as bass
import concourse.mybir as mybir
import concourse.tile as tile

# ---------------------------------------------------------------------------
# Workaround for this container's walrus build: any instruction carrying more
# than ONE sync-wait command is rejected ("Too many sync wait commands").
# Split the extras onto preceding NOPs on the same engine.
# ---------------------------------------------------------------------------

def _dedup_ldweights(instructions):
    """Drop a PE Ldweights identical to the previous one when only moving-only
    Matmults sit between (the PE keeps the stationary block until replaced).
    The dropped instruction's sem waits/updates merge into the next PE
    instruction. Other engines' instructions pass through untouched."""
    out, last_key = [], None
    pending_waits, pending_updates = [], []
    for inst in instructions:
        op = inst.get("opcode", "")
        eng = inst.get("engine", "")
        if eng != "PE":
            out.append(inst)
            continue
        if op == "Ldweights":
            key = orjson.dumps(inst.get("ins"))
            if key == last_key:
                si = inst.get("sync_info") or {}
                pending_waits.extend(si.get("on_wait") or [])
                pending_updates.extend(si.get("on_update") or [])
                continue
            last_key = key
        elif op == "Matmult":
            if len(inst.get("ins", [])) > 1:
                last_key = None          # self-loading matmul replaces weights
        elif op != "NoOp":
            last_key = None              # conservative: any other PE op resets
        if pending_waits or pending_updates:
            si = inst.setdefault("sync_info", {"on_update": [], "on_wait": []})
            si["on_wait"] = (si.get("on_wait") or []) + pending_waits
            si["on_update"] = (si.get("on_update") or []) + pending_updates
            pending_waits, pending_updates = [], []
        out.append(inst)
    assert not pending_waits and not pending_updates
    return out


def _legalize_bir_json(bir_json: bytes) -> bytes:
    bir = orjson.loads(bir_json)
    for fn in bir.get("functions", []):
        for bb in fn.get("blocks", []):
            bb["instructions"] = _dedup_ldweights(bb.get("instructions", []))
            out = []
            for inst in bb.get("instructions", []):
                si = inst.get("sync_info")
                waits = si.get("on_wait") if si else None
                if waits and len(waits) > 1:
                    for i, w in enumerate(waits[:-1]):
                        nop = {
                            "engine": inst["engine"], "ins": [], "outs": [],
                            "name": f"{inst['name']}_lw{i}", "opcode": "NoOp",
                            "sync_info": {"on_update": [], "on_wait": [w]},
                        }
                        if "debug" in inst:
                            nop["debug"] = inst["debug"]
                        out.append(nop)
                    si["on_wait"] = [waits[-1]]
                out.append(inst)
            bb["instructions"] = out
    return orjson.dumps(bir)


def _install_legalizer():
    import concourse.bass_utils as bu
    import concourse.bass2jax as b2j
    if getattr(bu.compile_bir_kernel, "_legalized", False):
        return
    orig = bu.compile_bir_kernel

    def wrapped(bir_json, tmpdir, neff_name="file.neff"):
        return orig(_legalize_bir_json(bytes(bir_json)), tmpdir, neff_name=neff_name)

    wrapped._legalized = True
    bu.compile_bir_kernel = wrapped
    b2j.compile_bir_kernel = wrapped


_install_legalizer()

# ---------------------------------------------------------------------------
# Jit-once SPMD runner over axon PJRT (run_bass_kernel_spmd re-jits per call).
# ---------------------------------------------------------------------------

class SpmdRunner:
    def __init__(self, nc, n_cores):
        import jax
        from jax.experimental.shard_map import shard_map
        from jax.sharding import Mesh, PartitionSpec
        import concourse.bass2jax as b2j
        b2j.install_neuronx_cc_hook()
        self.n_cores = n_cores
        partition_name = nc.partition_id_tensor.name if nc.partition_id_tensor else None
        in_names, out_names, out_avals = [], [], []
        for alloc in nc.m.functions[0].allocations:
            if not isinstance(alloc, mybir.MemoryLocationSet):
                continue
            name = alloc.memorylocations[0].name
            if alloc.kind == "ExternalInput":
                if name != partition_name:
                    in_names.append(name)
            elif alloc.kind == "ExternalOutput":
                out_names.append(name)
                out_avals.append(jax.core.ShapedArray(tuple(alloc.tensor_shape),
                                                      mybir.dt.np(alloc.dtype)))
        self.in_names, self.out_names, self.out_avals = in_names, out_names, out_avals
        n_params = len(in_names)

        def _body(*args):
            operands = list(args)
            if partition_name is not None:
                operands.append(b2j.partition_id_tensor())
            outs = b2j._bass_exec_p.bind(
                *operands,
                out_avals=tuple(out_avals),
                in_names=tuple(list(in_names) + list(out_names) +
                               ([partition_name] if partition_name else [])),
                out_names=tuple(out_names),
                lowering_input_output_aliases=(),
                sim_require_finite=False, sim_require_nnan=False, nc=nc,
            )
            return tuple(outs)

        devices = jax.devices()[:n_cores]
        assert len(devices) == n_cores, f"need {n_cores} cores, have {len(devices)}"
        mesh = Mesh(np.asarray(devices), ("core",))
        nz = len(out_names)
        self._fn = jax.jit(
            shard_map(_body, mesh=mesh,
                      in_specs=(PartitionSpec("core"),) * (n_params + nz),
                      out_specs=(PartitionSpec("core"),) * nz,
                      check_rep=False),
            keep_unused=True,
        )
        self._zeros = [
            jax.device_put(np.zeros((n_cores * a.shape[0], *a.shape[1:]), a.dtype))
            for a in out_avals
        ]
        self._jax = jax

    def put_inputs(self, in_maps):
        jax = self._jax
        concat = [
            np.concatenate([np.asarray(in_maps[c][n]) for c in range(self.n_cores)], axis=0)
            for n in self.in_names
        ]
        return [jax.device_put(a) for a in concat]

    def execute(self, dev):
        return self._fn(*dev, *self._zeros)

    def run(self, in_maps):
        dev = self.put_inputs(in_maps)
        outs = [np.asarray(o) for o in self.execute(dev)]
        return [
            {n: outs[i].reshape(self.n_cores, *self.out_avals[i].shape)[c]
             for i, n in enumerate(self.out_names)}
            for c in range(self.n_cores)
        ]


# ---------------------------------------------------------------------------
# Problem constants (hardcoded per the harness contract) and kernel builder.
# ---------------------------------------------------------------------------

D = 1024          # model dim
F = 2816          # expert hidden dim
E = 8             # experts == cores
TOPK = 2
DT = D // 128
FT = F // 128
FP32 = mybir.dt.float32
BF16 = mybir.dt.bfloat16


def _tok_tiles(C, maxw=512):
    """Split C into ceil(C/maxw) near-even tiles, all even-width (odd moving
    widths fail the matmul ISA check) and >=256 when C allows."""
    assert C % 2 == 0, C
    k = -(-C // maxw)
    half = C // 2
    base, rem = divmod(half, k)
    tiles, t0 = [], 0
    for i in range(k):
        tn = 2 * (base + (1 if i < rem else 0))
        tiles.append((t0, tn))
        t0 += tn
    return tiles


def build(C, n_copies=1):
    TOK = _tok_tiles(C)
    NTOK = len(TOK)
    nc = bass.Bass(target_bir_lowering=False)
    xt = nc.dram_tensor("xt", [D, C], BF16, kind="ExternalInput")
    w1p = nc.dram_tensor("w1p", [FT, 128, DT * 128], BF16, kind="ExternalInput")
    w2p = nc.dram_tensor("w2p", [FT, 128, DT * 128], BF16, kind="ExternalInput")
    w3p = nc.dram_tensor("w3p", [DT, 128, FT * 128], BF16, kind="ExternalInput")
    cw = nc.dram_tensor("cw", [128, C], FP32, kind="ExternalInput")
    yt = nc.dram_tensor("yt", [D, C], FP32, kind="ExternalOutput")

    with tile.TileContext(nc) as tc:
        with (
            tc.tile_pool(name="resident", bufs=1) as rpool,
            tc.tile_pool(name="stream", bufs=2) as spool,
            tc.tile_pool(name="work", bufs=2) as wpool,
            tc.tile_pool(name="psum", bufs=1, space="PSUM") as ppool,
        ):
            def emit_x_block():
                """Issue x/cw loads t-major; return the tiles."""
                xk = [[rpool.tile([128, TOK[ti][1]], BF16,
                                  tag=f"x{k}_{ti}", name=f"x{k}_{ti}")
                       for ti in range(NTOK)] for k in range(DT)]
                cwsb = rpool.tile([128, C], FP32, tag="cw", name="cwsb", bufs=2)
                for ti, (t0, tn) in enumerate(TOK):
                    for k in range(DT):
                        nc.sync.dma_start(out=xk[k][ti][:, :],
                                          in_=xt[k * 128:(k + 1) * 128, t0:t0 + tn])
                nc.sync.dma_start(out=cwsb[:, :], in_=cw[:, :])
                return xk, cwsb

            wscr = rpool.tile([128, 640], BF16, tag="wscr", name="wscr")
            pscr = ppool.tile([128, 512], FP32, tag="pscr", bufs=1, name="pscr")
            nc.vector.memset(wscr[:, :], 0.125)
            for _i in range(25):
                nc.tensor.matmul(pscr[:, :512], wscr[:, bass.ts(0, 128)],
                                 wscr[:, bass.ds(128, 512)], start=True, stop=True)

            xcur = emit_x_block()
            for _copy in range(n_copies):
                xk, cwsb = xcur

                # --- phase A: gT = silu((x@w1.T).T) * (x@w2.T).T ---
                gsb = rpool.tile([128, FT * C], BF16, tag="g", name="gsb")
                for f in range(FT):
                    w1sb = spool.tile([128, DT * 128], BF16, tag="w1sb")
                    w2sb = spool.tile([128, DT * 128], BF16, tag="w2sb")
                    nc.scalar.dma_start(out=w1sb[:, :], in_=w1p[f])
                    nc.scalar.dma_start(out=w2sb[:, :], in_=w2p[f])
                    h1 = [ppool.tile([128, 512], FP32, tag=f"h1_{ti}", bufs=1,
                                     name=f"h1_{ti}") for ti in range(NTOK)]
                    h2 = [ppool.tile([128, 512], FP32, tag=f"h2_{ti}", bufs=1,
                                     name=f"h2_{ti}") for ti in range(NTOK)]
                    if f == 0:
                        # t-outer: matches t-major x DMA arrival order
                        for ti, (t0, tn) in enumerate(TOK):
                            for hp, wsb in ((h1, w1sb), (h2, w2sb)):
                                for k in range(DT):
                                    nc.tensor.matmul(hp[ti][:, :tn],
                                                     wsb[:, bass.ts(k, 128)],
                                                     xk[k][ti][:, :],
                                                     start=(k == 0), stop=(k == DT - 1))
                    else:
                        # k-outer/t-inner: consecutive matmuls share weights
                        for hp, wsb in ((h1, w1sb), (h2, w2sb)):
                            for k in range(DT):
                                for ti, (t0, tn) in enumerate(TOK):
                                    nc.tensor.matmul(hp[ti][:, :tn],
                                                     wsb[:, bass.ts(k, 128)],
                                                     xk[k][ti][:, :],
                                                     start=(k == 0), stop=(k == DT - 1))
                    for ti, (t0, tn) in enumerate(TOK):
                        smu = wpool.tile([128, 512], FP32, tag=f"smu_{ti}")
                        nc.scalar.activation(smu[:, :tn], h1[ti][:, :tn],
                                             mybir.ActivationFunctionType.Silu)
                        nc.vector.tensor_mul(gsb[:, bass.ds(f * C + t0, tn)],
                                             smu[:, :tn], h2[ti][:, :tn])

                # prefetch next copy's x/cw while phase B runs
                if _copy + 1 < n_copies:
                    xcur = emit_x_block()

                # --- phase B: yT = (g@w3.T).T scaled by combine weight ---
                for d in range(DT):
                    w3sb = spool.tile([128, FT * 128], BF16, tag="w3sb")
                    nc.scalar.dma_start(out=w3sb[:, :], in_=w3p[d])
                    # reuse the h psum banks, alternating by d parity
                    hb = "h1" if d % 2 == 0 else "h2"
                    yp = [ppool.tile([128, 512], FP32, tag=f"{hb}_{ti}", bufs=1,
                                     name=f"yp_{ti}") for ti in range(NTOK)]
                    for f in range(FT):
                        for ti, (t0, tn) in enumerate(TOK):
                            nc.tensor.matmul(yp[ti][:, :tn], w3sb[:, bass.ts(f, 128)],
                                             gsb[:, bass.ds(f * C + t0, tn)],
                                             start=(f == 0), stop=(f == FT - 1))
                    for ti, (t0, tn) in enumerate(TOK):
                        osb = wpool.tile([128, 512], FP32, tag=f"osb_{ti}")
                        nc.vector.tensor_mul(osb[:, :tn], yp[ti][:, :tn],
                                             cwsb[:, t0:t0 + tn])
                        nc.sync.dma_start(out=yt[d * 128:(d + 1) * 128, t0:t0 + tn],
                                          in_=osb[:, :tn])
    return nc


# ---------------------------------------------------------------------------
# Host routing / dispatch / combine
# ---------------------------------------------------------------------------

def _route(x, gw):
    logits = x @ gw.T                                    # [T, E]
    order = np.argsort(-logits, axis=1, kind="stable")   # ties -> lower idx, as top_k
    idx = order[:, :TOPK]
    vals = np.take_along_axis(logits, idx, axis=1)
    ex = np.exp(vals - vals[:, :1])
    sv = ex / ex.sum(axis=1, keepdims=True)
    per_expert = []
    for e in range(E):
        mask = idx == e
        tok = np.nonzero(mask.any(axis=1))[0]
        per_expert.append((tok, sv[mask]))
    return per_expert


def _w1_image(w, np_dt):
    """Phase A SBUF image: out[f][p][k*128+m] = w[f*128+m, k*128+p]."""
    blk = w.reshape(FT, 128, DT, 128)                 # [f, m, k, p]
    return np.ascontiguousarray(blk.transpose(0, 3, 2, 1).reshape(FT, 128, DT * 128)
                                ).astype(np_dt)


def _w3_image(w3e, np_dt):
    """Phase B SBUF image: out[d][p][f*128+m] = w3[d*128+m, f*128+p]."""
    blk = w3e.reshape(DT, 128, FT, 128)               # [d, m, f, p]
    return np.ascontiguousarray(blk.transpose(0, 3, 2, 1).reshape(DT, 128, FT * 128)
                                ).astype(np_dt)


_runners = {}


def _get_runner(C):
    if C not in _runners:
        _runners[C] = SpmdRunner(build(C), E)
    return _runners[C]


def make_in_maps(x, gw, w1, w2, w3):
    import ml_dtypes
    np_dt = ml_dtypes.bfloat16
    per_expert = _route(x, gw)
    max_n = max(len(tok) for tok, _ in per_expert)
    C = max(256, max_n + (max_n % 2))
    in_maps = []
    for e in range(E):
        tok, w = per_expert[e]
        n = len(tok)
        xt = np.zeros((D, C), np_dt)
        xt[:, :n] = x[tok].astype(np_dt).T
        cwrow = np.zeros((1, C), np.float32)
        cwrow[0, :n] = w
        in_maps.append({
            "xt": xt,
            "w1p": _w1_image(w1[e], np_dt),
            "w2p": _w1_image(w2[e], np_dt),
            "w3p": _w3_image(w3[e], np_dt),
            "cw": np.ascontiguousarray(np.broadcast_to(cwrow, (128, C))),
        })
    return in_maps, per_expert, C


def kernel(xmat, gw, w1, w2, w3):
    B, L, d = xmat.shape
    x = np.ascontiguousarray(np.asarray(xmat, dtype=np.float32).reshape(-1, d))
    gw = np.asarray(gw, dtype=np.float32)
    w1 = np.asarray(w1, dtype=np.float32)
    w2 = np.asarray(w2, dtype=np.float32)
    w3 = np.asarray(w3, dtype=np.float32)

    in_maps, per_expert, C = make_in_maps(x, gw, w1, w2, w3)
    results = _get_runner(C).run(in_maps)

    y = np.zeros((x.shape[0], D), np.float32)
    for e in range(E):
        tok, _ = per_expert[e]
        y[tok] += results[e]["yt"][:, :len(tok)].T
    return y.reshape(B, L, d)
